# revision 5
# baseline (speedup 1.0000x reference)
"""AstroEconomicTransformer on 8 Trainium2 NeuronCores — TP-4 layout.

Sharding: cores 0-3 hold batch 0, cores 4-7 batch 1. Within each group of
4, Megatron-style tensor parallelism: core (rank g) owns attention heads
4g..4g+3 (256 q/k/v features), d_ff rows 1024g..1024(g+1), and — for the
residual stream, LayerNorms and collective staging — the token shard
256g..256(g+1) (sequence-parallel LN).

Every matmul streams a 512-wide moving operand (bf16) over the full
1024-token sequence: per layer each core runs ~600 matmuls of N=512
instead of ~2200 of N=256 (the PE sequencer is the scarce resource at
~71ns/instruction). Attention is entirely core-local: K/V for the owned
heads cover the whole sequence, so there is no K/V all-gather and no
DRAM round-trip; probabilities flow PSUM->SBUF(bf16)->PSUM.

Per layer: AllGather(x bf16) -> QKV -> attention (ones-augmented V rows
give softmax denominators; no max-subtraction, scores are O(1)) ->
Wo-partial -> ReduceScatter(bf16, token shards) -> residual+LN1 (own 256
tokens only) -> AllGather -> FFN (relu fused into PSUM eviction) ->
W2-partial -> ReduceScatter -> residual+LN2. The final LN + output head
run on the token shard; the (B,S,1) output is assembled on host.
"""

import os

import numpy as np

_SIM = os.environ.get("KSIM") == "1"  # local timeline-sim mode (no collectives)

B, S = 2, 1024
D, H, L, DFF = 1024, 16, 6, 4096
NM, NA, OUT = 10, 20, 1
HD = D // H
EPS = 1e-5

NCORES = 8
GPC = 4  # cores per batch group
T = S // GPC  # 256 tokens owned per core (residual/LN shard)
GROUPS = [[0, 1, 2, 3], [4, 5, 6, 7]]
P = 128
DT = D // P  # 8 feature tiles
HPC = H // GPC  # 4 heads per core
FS = HPC * HD  # 256 sharded q/k/v/ctx features
FTC = (DFF // GPC) // P  # 8 dff tiles per core
KT = S // P  # 8 key tiles
NH = 512  # moving-operand half width (one PSUM bank of fp32)

XELEM = DT * P * T  # bf16 elements in one token-shard x block

_RUNNER = None
REPS = int(os.environ.get("KREPS", "1"))


class _Cols:
    """Allocates columns in the (128, n) bias/constant matrix."""

    def __init__(self):
        self.cols = []

    def add(self, mat):  # mat: (128, n) -> first col index
        i = len(self.cols)
        self.cols.extend(np.asarray(mat, np.float32).T)
        return i

    def array(self):
        return np.stack(self.cols, axis=1).astype(np.float32)


def _grp(wT, rsl, csl, nk, nm):
    """(L, K, M) pre-transposed weight -> (L*nm, P, nk*P) SBUF tile images.

    Row (l, m) is the (128, nk*128) tile whose column block k is the lhsT
    block wT[l, rsl][k*128:(k+1)*128, csl][:, m*128:(m+1)*128].
    """
    sub = wT[:, rsl, :][:, :, csl]
    g = sub.reshape(L, nk, P, nm, P).transpose(0, 3, 2, 1, 4)
    return np.ascontiguousarray(g.reshape(L * nm, P, nk * P))


def _prep_host(inputs):
    f32 = np.float32
    g = {k: np.asarray(v, f32) for k, v in inputs.items()}
    import ml_dtypes

    bf = lambda a: np.ascontiguousarray(a).astype(ml_dtypes.bfloat16)
    tr = lambda w: w.transpose(0, 2, 1)

    shared = {
        "WmT": np.ascontiguousarray(g["Wm"].T),
        "WaT": np.ascontiguousarray(g["Wa"].T),
        "peT": np.ascontiguousarray(g["pe"][0].T),
        "WoutT": np.ascontiguousarray(g["Wout"].T),
        "onesb": np.ones((P, P), f32),
        "ident": bf(np.eye(P, dtype=f32)),
    }

    WqT, WkT, WvT = tr(g["Wq"]), tr(g["Wk"]), tr(g["Wv"])
    WoT, W1T, W2T = tr(g["Wo"]), tr(g["W1"]), tr(g["W2"])

    per_core = []
    idx = {}
    for c in range(NCORES):
        b, r = c // GPC, c % GPC
        fsh = slice(r * FS, (r + 1) * FS)  # feature shard (heads)
        dsh = slice(r * FTC * P, (r + 1) * FTC * P)  # dff shard
        al = slice(None)

        cols = _Cols()
        ix = {}
        bemb = np.concatenate([g["bm"], g["ba"]])
        ix["bemb"] = cols.add(bemb.reshape(DT, P).T)
        for l in range(L):
            ix[f"bq{l}"] = cols.add((g["bq"][l, fsh] * 0.125).reshape(2, P).T)
            ix[f"bk{l}"] = cols.add(g["bk"][l, fsh].reshape(2, P).T)
            ix[f"bv{l}"] = cols.add(g["bv"][l, fsh].reshape(2, P).T)
            ix[f"b1{l}"] = cols.add(g["b1"][l, dsh].reshape(FTC, P).T)
            ix[f"bo{l}"] = cols.add(g["bo"][l].reshape(DT, P).T)
            ix[f"b2{l}"] = cols.add(g["b2"][l].reshape(DT, P).T)
            ix[f"g1{l}"] = cols.add(g["ln1_g"][l].reshape(DT, P).T)
            ix[f"be1{l}"] = cols.add(g["ln1_b"][l].reshape(DT, P).T)
            ix[f"g2{l}"] = cols.add(g["ln2_g"][l].reshape(DT, P).T)
            ix[f"be2{l}"] = cols.add(g["ln2_b"][l].reshape(DT, P).T)
            ix[f"pb{l}"] = cols.add(
                np.tile(g["pbias"][l, r * HPC : (r + 1) * HPC][None, :], (P, 1))
            )
        ix["gf"] = cols.add(g["lnf_g"].reshape(DT, P).T)
        ix["bef"] = cols.add(g["lnf_b"].reshape(DT, P).T)
        ix["bout"] = cols.add(np.full((P, 1), g["bout"][0], f32))
        ix["eps"] = cols.add(np.full((P, 1), EPS, f32))
        bcols = cols.array()
        if c == 0:
            idx = ix
            idx["_nbc"] = bcols.shape[1]

        tsl = slice(r * T, (r + 1) * T)  # own token shard
        per_core.append(
            {
                "bcols": bcols,
                "mktT": np.ascontiguousarray(g["market_data"][b].T),
                "astT": np.ascontiguousarray(g["astro_data"][b].T),
                "mktTo": np.ascontiguousarray(g["market_data"][b, tsl].T),
                "astTo": np.ascontiguousarray(g["astro_data"][b, tsl].T),
                "peTo": np.ascontiguousarray(g["pe"][0, tsl].T),
                "Wq_g": bf(_grp(WqT, al, fsh, DT, 2)),
                "Wk_g": bf(_grp(WkT, al, fsh, DT, 2)),
                "Wv_g": bf(_grp(WvT, al, fsh, DT, 2)),
                "Wo_g": bf(_grp(WoT, fsh, al, 2, DT)),
                "W1_g": bf(_grp(W1T, al, dsh, DT, FTC)),
                "W2_g": bf(_grp(W2T, dsh, al, FTC, DT)),
            }
        )
    return shared, per_core, idx


# ---------------------------------------------------------------- device kernel
def _build(idx):
    from contextlib import ExitStack

    import concourse.mybir as mybir
    import concourse.tile as tile
    from concourse import bacc

    dt = mybir.dt
    F32, F32R, BF16 = dt.float32, dt.float32r, dt.bfloat16
    AF = mybir.ActivationFunctionType
    ALU = mybir.AluOpType

    nc = bacc.Bacc("TRN2", debug=False, num_devices=NCORES)

    NBC = idx["_nbc"]

    mktT = nc.declare_dram_parameter("mktT", [NM, S], F32R, isOutput=False)
    astT = nc.declare_dram_parameter("astT", [NA, S], F32R, isOutput=False)
    mktTo = nc.declare_dram_parameter("mktTo", [NM, T], F32R, isOutput=False)
    astTo = nc.declare_dram_parameter("astTo", [NA, T], F32R, isOutput=False)
    peTo = nc.declare_dram_parameter("peTo", [D, T], F32, isOutput=False)
    peT = nc.declare_dram_parameter("peT", [D, S], F32, isOutput=False)
    bcols_d = nc.declare_dram_parameter("bcols", [P, NBC], F32, isOutput=False)
    WmT = nc.declare_dram_parameter("WmT", [NM, D // 2], F32R, isOutput=False)
    WaT = nc.declare_dram_parameter("WaT", [NA, D // 2], F32R, isOutput=False)
    Wq_g = nc.declare_dram_parameter("Wq_g", [L * 2, P, DT * P], BF16, isOutput=False)
    Wk_g = nc.declare_dram_parameter("Wk_g", [L * 2, P, DT * P], BF16, isOutput=False)
    Wv_g = nc.declare_dram_parameter("Wv_g", [L * 2, P, DT * P], BF16, isOutput=False)
    Wo_g = nc.declare_dram_parameter("Wo_g", [L * DT, P, 2 * P], BF16, isOutput=False)
    W1_g = nc.declare_dram_parameter("W1_g", [L * FTC, P, DT * P], BF16, isOutput=False)
    W2_g = nc.declare_dram_parameter("W2_g", [L * DT, P, FTC * P], BF16, isOutput=False)
    WoutT = nc.declare_dram_parameter("WoutT", [D, OUT], F32R, isOutput=False)
    ones_d = nc.declare_dram_parameter("onesb", [P, P], F32R, isOutput=False)
    ident_d = nc.declare_dram_parameter("ident", [P, P], BF16, isOutput=False)
    y_out = nc.declare_dram_parameter("y", [1, T], F32, isOutput=True)

    rs_in = [nc.dram_tensor(f"rs_in{i}", [GPC, DT, P, T], BF16) for i in range(2 * L)]
    rs_out = [nc.dram_tensor(f"rs_out{i}", [DT, P, T], BF16) for i in range(2 * L)]
    ag_in = [nc.dram_tensor(f"ag_in{i}", [XELEM], BF16) for i in range(2 * L - 1)]
    ag_out = [nc.dram_tensor(f"ag_out{i}", [GPC, XELEM], BF16) for i in range(2 * L - 1)]

    with tile.TileContext(nc) as tc, ExitStack() as ctx:
        def pool(name, bufs, space="SBUF"):
            return ctx.enter_context(tc.tile_pool(name=name, bufs=bufs, space=space))

        singles = pool("singles", 1)
        pep = pool("pep", 2)  # pe f-tile staging
        xbp = pool("xbf", 2)  # bf16 full-seq activations (2MB each)
        xop = pool("xown", 3)  # fp32 own-token residual stream (1MB each)
        xobp = pool("xownb", 2)  # bf16 own-token (LN out for AG)
        qkvp = pool("qkv", 3)  # bf16 (P, 2*S) q/k/v
        vTp = pool("vT", 2)  # token-major ones-augmented v
        prp = pool("probs", 3)  # bf16 (P, S) attention probs
        cxp = pool("ctx", 2)  # bf16 (P, 2*S) normalized ctx
        ctp = pool("ctmp", 2)  # odd-head ctx staging
        hp = pool("harr", 1)  # bf16 (P, FTC*S) ffn hidden
        wp = pool("wrow", 4)  # bf16 (P, 1024) weight k-groups
        wop = pool("worow", 2)  # bf16 (P, 256) Wo k-groups
        rsp = pool("rsstage", 2)  # bf16 (P, S) partial-sum staging
        rlp = pool("rsload", 2)  # bf16 (P, DT*T) reduce-scatter result
        sqp = pool("sqp", 2)
        bcp = pool("bcp", 2)
        lntp = pool("lntp", 2)
        sp = pool("small", 3)
        recp = pool("recp", 2)
        bcbp = pool("bcbp", 2)
        embp = pool("embp", 2)

        psW = pool("psW", 2, space="PSUM")  # (P, 512) f32 matmul accumulators
        psC = pool("psC", 1, space="PSUM")  # (65, S) ctx accumulator
        psS = pool("psS", 4, space="PSUM")  # bcasts / transposes / LN stats

        bc = singles.tile([P, NBC], F32)
        nc.sync.dma_start(bc[:], bcols_d[:])
        onesb = singles.tile([P, P], F32R)
        nc.sync.dma_start(onesb[:], ones_d[:])
        ident = singles.tile([P, P], BF16)
        nc.sync.dma_start(ident[:], ident_d[:])
        wout_sb = singles.tile([P, DT], F32R)
        nc.sync.dma_start(
            wout_sb[:].rearrange("p (a o) -> p a o", o=OUT),
            WoutT[:].rearrange("(a p) o -> p a o", p=P),
        )
        in_sb = singles.tile([NA, 2 * S], F32R)
        nc.sync.dma_start(in_sb[0:NM, 0:S], mktT[:])
        nc.sync.dma_start(in_sb[0:NA, S : 2 * S], astT[:])
        wemb = singles.tile([NA, D // 2], F32R)
        nc.sync.dma_start(wemb[0:NM, :], WmT[:])
        wemb2 = singles.tile([NA, D // 2], F32R)
        nc.sync.dma_start(wemb2[:], WaT[:])

        def col(name, j=0, rows=P):
            return bc[0:rows, idx[name] + j : idx[name] + j + 1]

        def mm(out, lhsT, rhs, start, stop):
            nc.tensor.matmul(out, lhsT, rhs, start=start, stop=stop)

        import itertools

        _psctr = itertools.count()

        def psw2():
            return [psW.tile([P, NH], F32, tag="psW", name=f"psw{next(_psctr)}")
                    for _ in range(2)]

        def layernorm(src, gname, bname, dst):
            """src/dst: (P, DT*T) arrays, feature-major over own tokens."""
            s_ps = psS.tile([1, T], F32, tag="sm")
            s2_ps = psS.tile([1, T], F32, tag="sm")
            for m in range(DT):
                sq = sqp.tile([P, T], F32R, tag="sq")
                nc.vector.tensor_mul(sq[:], src[:, m * T : (m + 1) * T],
                                     src[:, m * T : (m + 1) * T])
                mm(s2_ps[:], onesb[:, 0:1], sq[:], start=(m == 0), stop=(m == DT - 1))
            for m in range(DT):
                mm(s_ps[:], onesb[:, 0:1], src[:, m * T : (m + 1) * T],
                   start=(m == 0), stop=(m == DT - 1))
            mu = sp.tile([1, T], F32R, tag="stat1")
            nc.vector.tensor_scalar_mul(mu[:], s_ps[:], 1.0 / D)
            ex2 = sp.tile([1, T], F32, tag="stat1")
            nc.vector.tensor_scalar_mul(ex2[:], s2_ps[:], 1.0 / D)
            mu2 = sp.tile([1, T], F32, tag="stat1")
            nc.vector.tensor_mul(mu2[:], mu[:], mu[:])
            var = sp.tile([1, T], F32, tag="stat1")
            nc.vector.tensor_sub(var[:], ex2[:], mu2[:])
            sd = sp.tile([1, T], F32, tag="stat1")
            nc.scalar.activation(sd[:], var[:], AF.Sqrt, bias=col("eps", rows=1), scale=1.0)
            rs = sp.tile([1, T], F32R, tag="stat1")
            with nc.allow_low_precision(reason="fp32r feeds the broadcast matmul"):
                nc.vector.reciprocal(rs[:], sd[:])
            mub_ps = psS.tile([P, T], F32, tag="sm")
            mm(mub_ps[:], onesb[0:1, :], mu[:], start=True, stop=True)
            rsb_ps = psS.tile([P, T], F32, tag="sm")
            mm(rsb_ps[:], onesb[0:1, :], rs[:], start=True, stop=True)
            mub = bcp.tile([P, T], F32, tag="bcast")
            nc.scalar.copy(mub[:], mub_ps[:])
            rsb = bcp.tile([P, T], F32, tag="bcast")
            nc.scalar.copy(rsb[:], rsb_ps[:])
            for m in range(DT):
                t1 = lntp.tile([P, T], F32, tag="lnt")
                nc.vector.tensor_sub(t1[:], src[:, m * T : (m + 1) * T], mub[:])
                t2 = lntp.tile([P, T], F32, tag="lnt")
                nc.vector.tensor_mul(t2[:], t1[:], rsb[:])
                nc.vector.tensor_scalar(
                    dst[:, m * T : (m + 1) * T], t2[:], col(gname, m), col(bname, m),
                    ALU.mult, ALU.add,
                )

        def collective(kind, op, src, dst):
            if _SIM:
                if kind == "AllGather":
                    for r in range(GPC):
                        nc.sync.dma_start(dst[r, :], src[:])
                else:  # ReduceScatter [GPC, DT, P, T] -> [DT, P, T]
                    nc.sync.dma_start(dst[:].opt(), src[0].opt())
            else:
                nc.gpsimd.collective_compute(
                    kind, op, replica_groups=GROUPS,
                    ins=[src[:].opt()], outs=[dst[:].opt()],
                )

        for _rep in range(REPS):
            rank = None  # token shard == collective rank == c % GPC (host side)

            # ======================================================== embed
            xb = xbp.tile([P, DT * S], BF16, tag="xbf", name="xb_emb")
            for m in range(DT):
                pe_t = pep.tile([P, S], F32, tag="pe")
                nc.sync.dma_start(
                    pe_t[:], peT[m * P : (m + 1) * P, :]
                )
                if m < 4:
                    w, nin, toff = wemb, NM, 0
                else:
                    w, nin, toff = wemb2, NA, S
                for hh in range(2):
                    pm = psW.tile([P, NH], F32, tag="psW")
                    mm(pm[:], w[0:nin, (m % 4) * P : (m % 4 + 1) * P],
                       in_sb[0:nin, toff + hh * NH : toff + (hh + 1) * NH],
                       start=True, stop=True)
                    nc.vector.scalar_tensor_tensor(
                        xb[:, m * S + hh * NH : m * S + (hh + 1) * NH],
                        pm[:], col("bemb", m), pe_t[:, hh * NH : (hh + 1) * NH],
                        ALU.add, ALU.add,
                    )
            # own-token residual from the pre-sliced per-core inputs (the
            # token shard is core-dependent; SPMD code can't slice x_full)
            ino = embp.tile([NA, 2 * T], F32R, tag="embin")
            nc.sync.dma_start(ino[0:NM, 0:T], mktTo[:])
            nc.sync.dma_start(ino[0:NA, T : 2 * T], astTo[:])
            x = xop.tile([P, DT * T], F32R, tag="xown", name="x_emb")
            for m in range(DT):
                pe_t = pep.tile([P, T], F32, tag="peo")
                nc.sync.dma_start(pe_t[:], peTo[m * P : (m + 1) * P, :])
                if m < 4:
                    w, nin, toff = wemb, NM, 0
                else:
                    w, nin, toff = wemb2, NA, T
                pm = psW.tile([P, NH], F32, tag="psW")
                mm(pm[0:P, 0:T], w[0:nin, (m % 4) * P : (m % 4 + 1) * P],
                   ino[0:nin, toff : toff + T], start=True, stop=True)
                nc.vector.scalar_tensor_tensor(
                    x[:, m * T : (m + 1) * T], pm[0:P, 0:T], col("bemb", m),
                    pe_t[:], ALU.add, ALU.add,
                )

            # ============================================================ layers
            for l in range(L):
                # ---- QKV projections (feature shard, full sequence)
                qt = qkvp.tile([P, 2 * S], BF16, tag="qkv", name=f"q{l}")
                kt = qkvp.tile([P, 2 * S], BF16, tag="qkv", name=f"k{l}")
                vt = qkvp.tile([P, 2 * S], BF16, tag="qkv", name=f"v{l}")
                for proj, wg_d, dst in (("q", Wq_g, qt), ("k", Wk_g, kt), ("v", Wv_g, vt)):
                    for m in range(2):
                        wrow = wp.tile([P, DT * P], BF16, tag="wrow")
                        nc.sync.dma_start(wrow[:], wg_d[l * 2 + m, :, :])
                        prs = psw2()
                        for k in range(DT):
                            for hh in range(2):
                                mm(prs[hh][:], wrow[:, k * P : (k + 1) * P],
                                   xb[:, k * S + hh * NH : k * S + (hh + 1) * NH],
                                   start=(k == 0), stop=(k == DT - 1))
                        for hh in range(2):
                            o = m * S + hh * NH
                            if proj == "q":
                                nc.vector.tensor_scalar(
                                    dst[:, o : o + NH], prs[hh][:], 0.125,
                                    col(f"bq{l}", m), ALU.mult, ALU.add)
                            else:
                                nc.vector.tensor_scalar_add(
                                    dst[:, o : o + NH], prs[hh][:],
                                    col(f"b{proj}{l}", m))

                # ---- v -> token-major ones-augmented rows
                vT = vTp.tile([P, KT * HPC * 65], BF16, tag="vT")
                vT3 = vT[:].rearrange("p (k c) -> p k c", c=65)
                nc.vector.memset(vT3[:, :, 64:65], 1.0)
                for tc_ in range(KT):
                    for m in range(2):
                        pT = psS.tile([P, P], BF16, tag="sm")
                        nc.tensor.transpose(
                            pT[:], vt[:, m * S + tc_ * P : m * S + (tc_ + 1) * P],
                            ident[:],
                        )
                        nc.scalar.copy(
                            vT3[:, tc_ * HPC + 2 * m : tc_ * HPC + 2 * m + 2, 0:64],
                            pT[:].rearrange("p (h c) -> p h c", c=64),
                        )

                # ---- attention per head
                ctxa = cxp.tile([P, 2 * S], BF16, tag="ctx")
                for h in range(HPC):
                    m, o = h // 2, 64 * (h % 2)
                    pc = psC.tile([65, S], F32, tag="psC")
                    for kt_ in range(KT):
                        probs = prp.tile([P, S], BF16, tag="probs")
                        pss = psw2()
                        for hh in range(2):
                            mm(pss[hh][:],
                               kt[o : o + 64, m * S + kt_ * P : m * S + (kt_ + 1) * P],
                               qt[o : o + 64, m * S + hh * NH : m * S + (hh + 1) * NH],
                               start=True, stop=True)
                        for hh in range(2):
                            nc.scalar.activation(
                                probs[:, hh * NH : (hh + 1) * NH], pss[hh][:],
                                AF.Exp, bias=col(f"pb{l}", h), scale=1.0)
                        for hh in range(2):
                            mm(pc[:, hh * NH : (hh + 1) * NH],
                               vT3[:, kt_ * HPC + h, :],
                               probs[:, hh * NH : (hh + 1) * NH],
                               start=(kt_ == 0), stop=(kt_ == KT - 1))
                    # normalize by the ones-row (softmax denominator)
                    rec = recp.tile([1, S], F32R, tag="rec")
                    with nc.allow_low_precision(reason="fp32r feeds broadcast mm"):
                        nc.vector.reciprocal(rec[:, 0:NH], pc[64:65, 0:NH])
                        nc.vector.reciprocal(rec[:, NH:S], pc[64:65, NH:S])
                    for hh in range(2):
                        pb_ps = psS.tile([64, NH], F32, tag="sm")
                        mm(pb_ps[:], onesb[0:1, 0:64],
                           rec[0:1, hh * NH : (hh + 1) * NH], start=True, stop=True)
                        bcsb = bcbp.tile([64, NH], F32, tag="bcsb")
                        nc.scalar.copy(bcsb[:], pb_ps[:])
                        if o == 0:
                            nc.vector.tensor_mul(
                                ctxa[0:64, m * S + hh * NH : m * S + (hh + 1) * NH],
                                pc[0:64, hh * NH : (hh + 1) * NH], bcsb[:])
                        else:
                            ctmp = ctp.tile([64, NH], BF16, tag="ctmp")
                            nc.vector.tensor_mul(
                                ctmp[:], pc[0:64, hh * NH : (hh + 1) * NH], bcsb[:])
                            nc.sync.dma_start(
                                ctxa[64:P, m * S + hh * NH : m * S + (hh + 1) * NH],
                                ctmp[:])

                # ---- Wo partial + ReduceScatter
                for m in range(DT):
                    worow = wop.tile([P, 2 * P], BF16, tag="worow")
                    nc.sync.dma_start(worow[:], Wo_g[l * DT + m, :, :])
                    rsst = rsp.tile([P, S], BF16, tag="rsst")
                    prs = psw2()
                    for k in range(2):
                        for hh in range(2):
                            mm(prs[hh][:], worow[:, k * P : (k + 1) * P],
                               ctxa[:, k * S + hh * NH : k * S + (hh + 1) * NH],
                               start=(k == 0), stop=(k == 1))
                    for hh in range(2):
                        nc.scalar.copy(rsst[:, hh * NH : (hh + 1) * NH], prs[hh][:])
                    nc.sync.dma_start(
                        rs_in[2 * l][:, m, :, :].rearrange("r p t -> p r t"),
                        rsst[:].rearrange("p (r t) -> p r t", r=GPC))
                collective("ReduceScatter", ALU.add, rs_in[2 * l], rs_out[2 * l])

                # ---- residual + LN1 (own tokens)
                rst = rlp.tile([P, DT * T], BF16, tag="rsload")
                nc.sync.dma_start(
                    rst[:].rearrange("p (f t) -> p f t", t=T),
                    rs_out[2 * l][:, :, :].rearrange("f p t -> p f t"))
                x1 = xop.tile([P, DT * T], F32R, tag="xown", name=f"x1_{l}")
                for m in range(DT):
                    nc.vector.scalar_tensor_tensor(
                        x1[:, m * T : (m + 1) * T], rst[:, m * T : (m + 1) * T],
                        col(f"bo{l}", m), x[:, m * T : (m + 1) * T],
                        ALU.add, ALU.add)
                x1ln = xop.tile([P, DT * T], F32R, tag="xown", name=f"x1ln_{l}")
                layernorm(x1[:], f"g1{l}", f"be1{l}", x1ln[:])
                x1b = xobp.tile([P, DT * T], BF16, tag="xownb")
                for m in range(DT):
                    nc.scalar.copy(x1b[:, m * T : (m + 1) * T],
                                   x1ln[:, m * T : (m + 1) * T])
                nc.sync.dma_start(
                    ag_in[2 * l][:].rearrange("(p n) -> p n", p=P), x1b[:])
                collective("AllGather", ALU.bypass, ag_in[2 * l], ag_out[2 * l])
                xb1 = xbp.tile([P, DT * S], BF16, tag="xbf", name=f"xb1_{l}")
                nc.sync.dma_start(
                    xb1[:].rearrange("p (f r t) -> p f r t", r=GPC, t=T),
                    ag_out[2 * l][:].rearrange("r (p f t) -> p f r t", p=P, f=DT))

                # ---- FFN1 (dff shard, full sequence), relu fused
                ht = hp.tile([P, FTC * S], BF16, tag="harr")
                for mf in range(FTC):
                    wrow = wp.tile([P, DT * P], BF16, tag="wrow")
                    nc.sync.dma_start(wrow[:], W1_g[l * FTC + mf, :, :])
                    prs = psw2()
                    for k in range(DT):
                        for hh in range(2):
                            mm(prs[hh][:], wrow[:, k * P : (k + 1) * P],
                               xb1[:, k * S + hh * NH : k * S + (hh + 1) * NH],
                               start=(k == 0), stop=(k == DT - 1))
                    for hh in range(2):
                        nc.scalar.activation(
                            ht[:, mf * S + hh * NH : mf * S + (hh + 1) * NH],
                            prs[hh][:], AF.Relu, bias=col(f"b1{l}", mf), scale=1.0)

                # ---- FFN2 partial + ReduceScatter
                for m in range(DT):
                    wrow = wp.tile([P, FTC * P], BF16, tag="wrow")
                    nc.sync.dma_start(wrow[:], W2_g[l * DT + m, :, :])
                    rsst = rsp.tile([P, S], BF16, tag="rsst")
                    prs = psw2()
                    for k in range(FTC):
                        for hh in range(2):
                            mm(prs[hh][:], wrow[:, k * P : (k + 1) * P],
                               ht[:, k * S + hh * NH : k * S + (hh + 1) * NH],
                               start=(k == 0), stop=(k == FTC - 1))
                    for hh in range(2):
                        nc.scalar.copy(rsst[:, hh * NH : (hh + 1) * NH], prs[hh][:])
                    nc.sync.dma_start(
                        rs_in[2 * l + 1][:, m, :, :].rearrange("r p t -> p r t"),
                        rsst[:].rearrange("p (r t) -> p r t", r=GPC))
                collective("ReduceScatter", ALU.add, rs_in[2 * l + 1], rs_out[2 * l + 1])

                # ---- residual + LN2
                rst2 = rlp.tile([P, DT * T], BF16, tag="rsload")
                nc.sync.dma_start(
                    rst2[:].rearrange("p (f t) -> p f t", t=T),
                    rs_out[2 * l + 1][:, :, :].rearrange("f p t -> p f t"))
                x2 = xop.tile([P, DT * T], F32R, tag="xown", name=f"x2_{l}")
                for m in range(DT):
                    nc.vector.scalar_tensor_tensor(
                        x2[:, m * T : (m + 1) * T], rst2[:, m * T : (m + 1) * T],
                        col(f"b2{l}", m), x1ln[:, m * T : (m + 1) * T],
                        ALU.add, ALU.add)
                x2ln = xop.tile([P, DT * T], F32R, tag="xown", name=f"x2ln_{l}")
                layernorm(x2[:], f"g2{l}", f"be2{l}", x2ln[:])
                x = x2ln
                if l < L - 1:
                    x2b = xobp.tile([P, DT * T], BF16, tag="xownb")
                    for m in range(DT):
                        nc.scalar.copy(x2b[:, m * T : (m + 1) * T],
                                       x2ln[:, m * T : (m + 1) * T])
                    nc.sync.dma_start(
                        ag_in[2 * l + 1][:].rearrange("(p n) -> p n", p=P), x2b[:])
                    collective("AllGather", ALU.bypass, ag_in[2 * l + 1],
                               ag_out[2 * l + 1])
                    xb = xbp.tile([P, DT * S], BF16, tag="xbf", name=f"xb_{l + 1}")
                    nc.sync.dma_start(
                        xb[:].rearrange("p (f r t) -> p f r t", r=GPC, t=T),
                        ag_out[2 * l + 1][:].rearrange("r (p f t) -> p f r t",
                                                       p=P, f=DT))

            # ============================================================ head
            xf = xop.tile([P, DT * T], F32R, tag="xown", name="xf")
            layernorm(x[:], "gf", "bef", xf[:])
            pyf = psS.tile([1, T], F32, tag="sm")
            for m in range(DT):
                mm(pyf[:], wout_sb[:, m : m + 1], xf[:, m * T : (m + 1) * T],
                   start=(m == 0), stop=(m == DT - 1))
            ysb = sp.tile([1, T], F32, tag="stat1")
            nc.scalar.activation(ysb[:], pyf[:], AF.Identity,
                                 bias=col("bout", 0, rows=1), scale=1.0)
            nc.sync.dma_start(y_out[:], ysb[:])

    nc.compile()
    return nc


# ---------------------------------------------------------------- runner
_SHARED_NAMES = frozenset(["WmT", "WaT", "peT", "WoutT", "onesb", "ident"])


def _make_runner(nc):
    import jax
    import concourse.mybir as mybir
    from concourse import bass2jax
    from jax.sharding import Mesh, PartitionSpec
    from jax.experimental.shard_map import shard_map

    bass2jax.install_neuronx_cc_hook()

    partition_name = nc.partition_id_tensor.name if nc.partition_id_tensor else None
    in_names, out_names, out_avals = [], [], []
    for alloc in nc.m.functions[0].allocations:
        if not isinstance(alloc, mybir.MemoryLocationSet):
            continue
        name = alloc.memorylocations[0].name
        if alloc.kind == "ExternalInput":
            if name != partition_name:
                in_names.append(name)
        elif alloc.kind == "ExternalOutput":
            out_names.append(name)
            out_avals.append(
                jax.core.ShapedArray(tuple(alloc.tensor_shape), mybir.dt.np(alloc.dtype))
            )
    n_params = len(in_names)
    n_outs = len(out_avals)
    all_in = in_names + out_names + ([partition_name] if partition_name else [])
    donate = tuple(range(n_params, n_params + n_outs))

    def _body(*args):
        operands = list(args)
        if partition_name is not None:
            operands.append(bass2jax.partition_id_tensor())
        return tuple(
            bass2jax._bass_exec_p.bind(
                *operands,
                out_avals=tuple(out_avals),
                in_names=tuple(all_in),
                out_names=tuple(out_names),
                lowering_input_output_aliases=(),
                sim_require_finite=True,
                sim_require_nnan=True,
                nc=nc,
            )
        )

    from jax.sharding import NamedSharding

    devices = jax.devices()[:NCORES]
    mesh = Mesh(np.asarray(devices), ("core",))
    repl_sharding = NamedSharding(mesh, PartitionSpec(None))
    core_sharding = NamedSharding(mesh, PartitionSpec("core"))
    in_specs = tuple(
        PartitionSpec(None) if name in _SHARED_NAMES else PartitionSpec("core")
        for name in in_names
    ) + (PartitionSpec("core"),) * n_outs
    out_specs = (PartitionSpec("core"),) * n_outs
    sharded = jax.jit(
        shard_map(_body, mesh=mesh, in_specs=in_specs, out_specs=out_specs,
                  check_rep=False),
        donate_argnums=donate,
        keep_unused=True,
    )

    class Runner:
        def upload(self, shared, per_core):
            ins = []
            for name in in_names:
                if name in _SHARED_NAMES:
                    ins.append((np.asarray(shared[name]), repl_sharding))
                else:
                    ins.append(
                        (
                            np.concatenate(
                                [np.asarray(per_core[c][name])
                                 for c in range(NCORES)],
                                axis=0,
                            ),
                            core_sharding,
                        )
                    )
            self.in_dev = [jax.device_put(a, s) for a, s in ins]
            jax.block_until_ready(self.in_dev)

        def dispatch(self):
            zeros = [
                jax.device_put(
                    np.zeros((NCORES * av.shape[0], *av.shape[1:]), av.dtype),
                    core_sharding,
                )
                for av in out_avals
            ]
            return sharded(*self.in_dev, *zeros)

        def collect(self, out_arrs):
            return [
                {
                    name: np.asarray(out_arrs[i]).reshape(NCORES, *out_avals[i].shape)[c]
                    for i, name in enumerate(out_names)
                }
                for c in range(NCORES)
            ]

        def run(self):
            import jax as _jax

            out_arrs = self.dispatch()
            _jax.block_until_ready(out_arrs)
            return self.collect(out_arrs)

    return Runner()


def get_runner(inputs):
    """Build (once) and return the runner with inputs uploaded."""
    global _RUNNER
    shared, per_core, idx = _prep_host(inputs)
    if _RUNNER is None:
        nc = _build(idx)
        _RUNNER = _make_runner(nc)
    _RUNNER.upload(shared, per_core)
    return _RUNNER


def kernel(**inputs) -> np.ndarray:
    runner = get_runner(inputs)
    res = runner.run()
    out = np.zeros((B, S, OUT), np.float32)
    for c in range(NCORES):
        b, chunk = c // GPC, c % GPC
        out[b, chunk * T : (chunk + 1) * T, 0] = res[c]["y"][0]
    return out
